# revision 1
# baseline (speedup 1.0000x reference)
"""GNN message passing (2-layer GCN-ish + dense similarity) on 8 trn2 NeuronCores.

Sharding: nodes row-partitioned across 8 cores (1024 rows each); edges
partitioned by destination.  Per layer: row-normalize own rows (fp32),
AllGather normalized features (fp16), per-core spmm as dedup-gather +
one-hot scatter matmuls (fp16, fp32 PSUM accum), Linear in fp32r, ELU.
Final: L2-normalize, AllGather emb^T, row-sharded emb @ emb^T with ReLU.
"""
import sys

sys.path.insert(0, "/opt/trn_rl_repo")

import numpy as np
import ml_dtypes  # noqa: F401  (bf16/fp16 numpy dtypes)

import concourse.bass as bass
import concourse.bacc as bacc
import concourse.mybir as mybir
from concourse import tile
from concourse.tile import add_dep_helper
from concourse import library_config
from concourse.bass_utils import run_bass_kernel_spmd

N = 8192        # nodes
D = 512         # feature dim
C = 8           # cores
NL = N // C     # nodes per core (1024)
NG = 4          # dest groups per core
GD = NL // NG   # dests per group (256)
NSG = NG * 2    # gather subgroups per core (half-groups)

f32 = mybir.dt.float32
f32r = mybir.dt.float32r
f16 = mybir.dt.float16
i16 = mybir.dt.int16

_compiled: dict[int, object] = {}
DEBUG = False
ABLATE: set = set()


def _build(MCH: int, timing: bool = False):
    """Build + finalize the SPMD program for MCH gather-chunks per subgroup.

    timing=True replaces collectives with equivalent-volume local DMAs so the
    program is single-core simulatable (TimelineSim) for cost-model profiling.
    """
    nc = bacc.Bacc("TRN2", target_bir_lowering=False, debug=False, num_devices=C)

    xloc = nc.declare_dram_parameter("xloc", [NL, D], f32, isOutput=False)
    gidx = nc.declare_dram_parameter("gidx", [128, NSG, MCH * 8], i16, isOutput=False)
    sblk = nc.declare_dram_parameter("sblk", [NSG, 128, MCH, GD], f16, isOutput=False)
    wt = nc.declare_dram_parameter("wt", [128, 4, 4, 128], f32r, isOutput=False)
    bcol = nc.declare_dram_parameter("bcol", [128, 4], f32, isOutput=False)
    brow = nc.declare_dram_parameter("brow", [1, 1024], f32r, isOutput=False)
    out = nc.declare_dram_parameter("out", [NL, N], f16, isOutput=True)
    if DEBUG:
        dbg_xn0 = nc.declare_dram_parameter("dbg_xn0", [NL, D], f32, isOutput=True)
        dbg_agg = nc.declare_dram_parameter("dbg_agg", [128, 4, GD], f32, isOutput=True)
        dbg_x1T = nc.declare_dram_parameter("dbg_x1T", [128, 4, NL], f32, isOutput=True)
        dbg_x1r = nc.declare_dram_parameter("dbg_x1r", [128, C, D], f32, isOutput=True)
        dbg_emb = nc.declare_dram_parameter("dbg_emb", [128, C, D], f32, isOutput=True)
        dbg_xn1 = nc.declare_dram_parameter("dbg_xn1", [128, C, D], f32, isOutput=True)
        dbg_agg2 = nc.declare_dram_parameter("dbg_agg2", [128, 4, GD], f32, isOutput=True)
        dbg_x2r = nc.declare_dram_parameter("dbg_x2r", [128, C, D], f32, isOutput=True)
        dbg_G0 = nc.declare_dram_parameter("dbg_G0", [128, MCH, D], f16, isOutput=True)
        dbg_G1 = nc.declare_dram_parameter("dbg_G1", [128, MCH, D], f16, isOutput=True)

    NIDX = MCH * 128
    Act = mybir.ActivationFunctionType
    Alu = mybir.AluOpType
    start_fcs = {fc for fc in range(4) if (fc * GD * 4) % 2048 == 0}
    stop_fcs = {fc for fc in range(4) if ((fc + 1) * GD * 4) % 2048 == 0 or fc == 3}

    with tile.TileContext(nc) as tc:
        nc.gpsimd.load_library(library_config.mlp)
        with (
            tc.tile_pool(name="persist", bufs=1) as pp,
            tc.tile_pool(name="dram", bufs=1, space="DRAM") as dram,
        ):
            # persistent SBUF state
            idx_sb = pp.tile([128, NSG, MCH * 8], i16)
            wt_sb = pp.tile([128, 4, 4, 128], f32r)
            bc_sb = pp.tile([128, 4], f32)
            br_sb = pp.tile([1, 1024], f32r)
            embT_own = pp.tile([128, 4, NL], f16)
            nc.sync.dma_start(out=idx_sb[:], in_=gidx[:])
            nc.sync.dma_start(out=wt_sb[:], in_=wt[:])
            nc.sync.dma_start(out=bc_sb[:], in_=bcol[:])
            nc.sync.dma_start(out=br_sb[:], in_=brow[:])

            # DRAM internals / collective buffers
            ag_in = [dram.tile([NL, D], f16, name=f"agin{l}") for l in range(2)]
            shr = "Local" if timing else "Shared"
            xfull = [
                dram.tile([N, D], f16, addr_space=shr, name=f"xfull{l}")
                for l in range(2)
            ]
            agT_in = dram.tile([D, NL], f16)
            embT_full = dram.tile([C * D, NL], f16, addr_space=shr)

            rg = [list(range(C))]

            with (
                tc.tile_pool(name="gpool", bufs=3) as gpool,
                tc.tile_pool(name="spool", bufs=3) as spool,
                tc.tile_pool(name="xrow", bufs=2) as xrow,
                tc.tile_pool(name="tmp", bufs=2) as tmp,
                tc.tile_pool(name="dbgp", bufs=1) as dbgp,
                tc.tile_pool(name="psA", bufs=2, space="PSUM") as psA,
                tc.tile_pool(name="psH", bufs=2, space="PSUM") as psH,
            ):
                # ---- phase 0: normalize own rows of x in fp32, AG to xfull[0]
                x0 = xrow.tile([128, C, D], f32, tag="x0", bufs=1)
                nc.sync.dma_start(
                    out=x0[:], in_=xloc.rearrange("(s p) f -> p s f", p=128)
                )
                s0 = tmp.tile([128, C], f32, tag="rs")
                nc.vector.tensor_reduce(
                    out=s0[:], in_=x0[:], axis=mybir.AxisListType.X, op=Alu.add
                )
                nc.vector.tensor_scalar_add(s0[:], s0[:], 1e-4)
                r0 = tmp.tile([128, C], f32, tag="rr")
                nc.vector.reciprocal(r0[:], s0[:])
                xn0 = xrow.tile([128, C, D], f16, tag="xn")
                for s in range(C):
                    nc.vector.tensor_scalar_mul(
                        xn0[:, s, :], x0[:, s, :], r0[:, s : s + 1]
                    )
                nc.sync.dma_start(
                    out=ag_in[0].rearrange("(s p) f -> p s f", p=128), in_=xn0[:]
                )
                cc = [None, None]

                def all_gather(src_t, dst_t, rows):
                    if timing:
                        last = None
                        for r in range(2):
                            last = nc.sync.dma_start(
                                out=dst_t[r * rows : (r + 1) * rows], in_=src_t[:]
                            )
                        return last
                    return nc.gpsimd.collective_compute(
                        "AllGather",
                        Alu.bypass,
                        ins=[src_t.opt()],
                        outs=[dst_t.opt()],
                        replica_groups=rg,
                    )

                cc[0] = all_gather(ag_in[0], xfull[0], NL)
                if DEBUG:
                    dxn = dbgp.tile([128, C, D], f32, tag="dxn")
                    nc.vector.tensor_copy(dxn[:], xn0[:])
                    nc.sync.dma_start(
                        out=dbg_xn0.rearrange("(s p) f -> p s f", p=128), in_=dxn[:]
                    )

                for layer in range(2):
                    src = xfull[layer]
                    xT = xrow.tile([128, 4, NL], f16, tag="xT")
                    xr = xrow.tile([128, C, D], f16, tag="xr")
                    xn1 = xrow.tile([128, C, D], f16, tag="xn")
                    s1 = tmp.tile([128, C], f32, tag="rs")
                    r1 = tmp.tile([128, C], f32, tag="rr")
                    sqt = tmp.tile([128, D], f32, tag="sqt")
                    for g in range(NG):
                        aggT = psA.tile([128, 4, GD], f32, tag="aggT")
                        for h in range(2):
                            sg = g * 2 + h
                            G = gpool.tile([128, MCH, D], f16, tag="G")
                            if "gather" in ABLATE:
                                gi = None
                            else:
                                gi = nc.gpsimd.dma_gather(
                                G[:], src[:], idx_sb[:, sg, :], NIDX, NIDX, D,
                                single_packet=False,
                            )
                            if gi is not None and not timing:
                                add_dep_helper(
                                    gi.ins, cc[layer].ins, sync=True,
                                    reason="gather reads AG output",
                                )
                            if DEBUG and layer == 0 and g == 0:
                                nc.sync.dma_start(
                                    out=(dbg_G0 if h == 0 else dbg_G1)[:], in_=G[:]
                                )
                            S = spool.tile([128, MCH, GD], f16, tag="S")
                            if "sdma" not in ABLATE:
                                nc.sync.dma_start(out=S[:], in_=sblk[sg])
                            for c in range(MCH if "spmm" not in ABLATE else 0):
                                first = h == 0 and c == 0
                                last = h == 1 and c == MCH - 1
                                for fc in range(4):
                                    # start/stop once per PSUM bank (2KB zero
                                    # region = two fc slices)
                                    nc.tensor.matmul(
                                        aggT[:, fc, :],
                                        lhsT=G[:, c, fc * 128 : (fc + 1) * 128],
                                        rhs=S[:, c, :],
                                        start=first and fc in start_fcs,
                                        stop=last and fc in stop_fcs,
                                    )
                        # aggT (PSUM f32) -> SBUF f32, then Linear in fp32r
                        aggs = tmp.tile([128, 4, GD], f32r, tag="aggs")
                        nc.scalar.copy(out=aggs[:], in_=aggT[:])
                        if DEBUG and layer == 0 and g == 0:
                            nc.sync.dma_start(out=dbg_agg[:], in_=aggs[:].bitcast(f32))
                        if DEBUG and layer == 1 and g == 0:
                            nc.sync.dma_start(out=dbg_agg2[:], in_=aggs[:].bitcast(f32))
                        hT = psH.tile([128, 4, GD], f32, tag="hT")
                        for fo in range(4):
                            for fi in range(4):
                                nc.tensor.matmul(
                                    hT[:, fo, :],
                                    lhsT=wt_sb[:, fi, fo, :],
                                    rhs=aggs[:, fi, :],
                                    start=(fi == 0 and fo in start_fcs),
                                    stop=False,
                                )
                            # bias: rank-1 update b_row[fo] x ones
                            nc.tensor.matmul(
                                hT[:, fo, :],
                                lhsT=br_sb[:, fo * 128 : (fo + 1) * 128],
                                rhs=br_sb[:, 512 : 512 + GD],
                                start=False,
                                stop=(fo in stop_fcs),
                            )
                        # ELU(hT) -> xT[:, :, g*GD:(g+1)*GD] (fp16), whole group
                        neg = tmp.tile([128, 4, GD], f32, tag="neg", bufs=1)
                        nc.vector.tensor_scalar_min(neg[:], hT[:], 0.0)
                        en = tmp.tile([128, 4, GD], f32, tag="en", bufs=1)
                        nc.scalar.activation(en[:], neg[:], Act.Exp)
                        pos = tmp.tile([128, 4, GD], f32, tag="pos", bufs=1)
                        nc.vector.tensor_scalar_max(pos[:], hT[:], 0.0)
                        nc.vector.tensor_tensor(
                            out=pos[:], in0=pos[:], in1=en[:], op=Alu.add
                        )
                        nc.vector.tensor_scalar_add(
                            xT[:, :, g * GD : (g + 1) * GD], pos[:], -1.0
                        )
                        # ---- per-group tail: transpose to row-major + normalize
                        sl0 = g * (GD // 128)
                        nsl = GD // 128
                        for fo in range(4):
                            nc.sync.dma_start(
                                out=xr[:, sl0 : sl0 + nsl, fo * 128 : (fo + 1) * 128],
                                in_=xT[:, fo, g * GD : (g + 1) * GD],
                                transpose=True,
                            )
                        if layer == 0:
                            nc.vector.tensor_reduce(
                                out=s1[:, sl0 : sl0 + nsl],
                                in_=xr[:, sl0 : sl0 + nsl, :],
                                axis=mybir.AxisListType.X,
                                op=Alu.add,
                            )
                            nc.vector.tensor_scalar_add(
                                s1[:, sl0 : sl0 + nsl], s1[:, sl0 : sl0 + nsl], 1e-4
                            )
                            nc.vector.reciprocal(
                                r1[:, sl0 : sl0 + nsl], s1[:, sl0 : sl0 + nsl]
                            )
                            for sl in range(sl0, sl0 + nsl):
                                nc.vector.tensor_scalar_mul(
                                    xn1[:, sl, :], xr[:, sl, :], r1[:, sl : sl + 1]
                                )
                            nc.sync.dma_start(
                                out=ag_in[1].rearrange("(s p) f -> p s f", p=128)[
                                    :, sl0 : sl0 + nsl, :
                                ],
                                in_=xn1[:, sl0 : sl0 + nsl, :],
                            )
                        else:
                            for sl in range(sl0, sl0 + nsl):
                                nc.scalar.activation(
                                    sqt[:],
                                    xr[:, sl, :],
                                    Act.Square,
                                    accum_out=s1[:, sl : sl + 1],
                                )
                            nc.vector.tensor_scalar_max(
                                s1[:, sl0 : sl0 + nsl], s1[:, sl0 : sl0 + nsl], 1e-24
                            )
                            nc.scalar.activation(
                                s1[:, sl0 : sl0 + nsl],
                                s1[:, sl0 : sl0 + nsl],
                                Act.Sqrt,
                            )
                            nc.vector.reciprocal(
                                r1[:, sl0 : sl0 + nsl], s1[:, sl0 : sl0 + nsl]
                            )
                            for sl in range(sl0, sl0 + nsl):
                                nc.vector.tensor_scalar_mul(
                                    xn1[:, sl, :], xr[:, sl, :], r1[:, sl : sl + 1]
                                )
                            for sl in range(sl0, sl0 + nsl):
                                nc.sync.dma_start(
                                    out=embT_own[:, :, sl * 128 : (sl + 1) * 128],
                                    in_=xn1[:, sl, :],
                                    transpose=True,
                                )
                            nc.sync.dma_start(
                                out=agT_in.rearrange("(s p) n -> p s n", p=128)[
                                    :, :, g * GD : (g + 1) * GD
                                ],
                                in_=embT_own[:, :, g * GD : (g + 1) * GD],
                            )
                    # per-group tail is emitted inside the group loop above
                    if layer == 0:
                        cc[1] = all_gather(ag_in[1], xfull[1], NL)
                    else:
                        cc_emb = all_gather(agT_in, embT_full, D)

            # ---- final: out = relu(emb_own @ emb_full^T), row-sharded
            with (
                tc.tile_pool(name="fin", bufs=1) as fin,
                tc.tile_pool(name="ob", bufs=4) as obp,
                tc.tile_pool(name="psF", bufs=2, space="PSUM") as psF,
            ):
                embT_all = fin.tile([128, 4, N], f16)
                for r in range(C):
                    ld = nc.sync.dma_start(
                        out=embT_all[:, :, r * NL : (r + 1) * NL],
                        in_=embT_full[r * D : (r + 1) * D].rearrange(
                            "(s p) n -> p s n", p=128
                        ),
                    )
                    add_dep_helper(
                        ld.ins, cc_emb.ins, sync=True,
                        reason="embT load reads AG output",
                    )
                for m in range(8 if "final" not in ABLATE else 0):
                    for nq in range(4):
                        ops = psF.tile([128, 4, 512], f32, tag="ops")
                        for fc in range(4):
                            for j in range(4):
                                nb = nq * 4 + j
                                nc.tensor.matmul(
                                    ops[:, j, :],
                                    lhsT=embT_own[:, fc, m * 128 : (m + 1) * 128],
                                    rhs=embT_all[:, fc, nb * 512 : (nb + 1) * 512],
                                    start=(fc == 0),
                                    stop=(fc == 3),
                                )
                        for j in range(4):
                            nb = nq * 4 + j
                            ob = obp.tile([128, 512], f16, tag="ob")
                            nc.scalar.activation(ob[:], ops[:, j, :], Act.Relu)
                            nc.sync.dma_start(
                                out=out[
                                    m * 128 : (m + 1) * 128,
                                    nb * 512 : (nb + 1) * 512,
                                ],
                                in_=ob[:],
                            )

    nc.finalize()
    return nc


def _preprocess(x, edge_index, edge_weight):
    """Per-core gather indices + one-hot scatter blocks (dedup per dest-group)."""
    row = edge_index[0].astype(np.int64)
    col = edge_index[1].astype(np.int64)
    w = edge_weight.astype(np.float32)

    per_core = []
    max_chunks = 1
    for k in range(C):
        msk = (row >= k * NL) & (row < (k + 1) * NL)
        rk = row[msk] - k * NL
        ck = col[msk]
        wk = w[msk]
        groups = []
        for g in range(NG):
            m2 = (rk >= g * GD) & (rk < (g + 1) * GD)
            rg_ = rk[m2] - g * GD
            cg = ck[m2]
            wg = wk[m2]
            uniq, inv = np.unique(cg, return_inverse=True)
            groups.append((uniq, inv, rg_, wg))
            max_chunks = max(max_chunks, -(-len(uniq) // 128))
        per_core.append(groups)

    MCH = -(-max_chunks // 2)  # chunks per half-group
    in_maps = []
    for k in range(C):
        gidx_k = np.zeros((128, NSG, MCH * 8), np.int16)
        sblk_k = np.zeros((NSG, 128, MCH, GD), np.float16)
        for g in range(NG):
            uniq, inv, rg_, wg = per_core[k][g]
            nu = len(uniq)
            Sf = np.zeros((2 * MCH * 128, GD), np.float32)
            np.add.at(Sf, (inv, rg_), wg)
            Sf = Sf.astype(np.float16).reshape(2 * MCH, 128, GD)
            idx_full = np.zeros(2 * MCH * 128, np.int16)
            idx_full[:nu] = uniq.astype(np.int16)
            for h in range(2):
                sg = g * 2 + h
                sblk_k[sg] = Sf[h * MCH : (h + 1) * MCH].transpose(1, 0, 2)
                sl = idx_full[h * MCH * 128 : (h + 1) * MCH * 128]
                w16 = sl.reshape(MCH * 8, 16).T  # [16, MCH*8], j = s*16+p
                gidx_k[:, sg, :] = np.tile(w16, (8, 1))
        in_maps.append({"gidx": gidx_k, "sblk": sblk_k})
    return in_maps, MCH


def kernel(x, edge_index, edge_weight, W, b):
    x = np.asarray(x, dtype=np.float32)
    edge_index = np.asarray(edge_index)
    edge_weight = np.asarray(edge_weight, dtype=np.float32)
    W = np.asarray(W, dtype=np.float32)
    b = np.asarray(b, dtype=np.float32)

    in_maps, MCH = _preprocess(x, edge_index, edge_weight)
    wt = np.ascontiguousarray(
        W.T.reshape(4, 128, 4, 128).transpose(1, 0, 2, 3)
    ).astype(np.float32)
    bc = np.ascontiguousarray(b.reshape(4, 128).T).astype(np.float32)
    br = np.concatenate([b, np.ones(512, np.float32)]).reshape(1, 1024).astype(np.float32)
    for k in range(C):
        in_maps[k]["xloc"] = np.ascontiguousarray(x[k * NL : (k + 1) * NL])
        in_maps[k]["wt"] = wt
        in_maps[k]["bcol"] = bc
        in_maps[k]["brow"] = br

    nc = _compiled.get(MCH)
    if nc is None:
        nc = _build(MCH)
        _compiled[MCH] = nc
    res = run_bass_kernel_spmd(nc, in_maps, list(range(C)))
    return np.concatenate(
        [res.results[k]["out"] for k in range(C)], axis=0
    ).astype(np.float32)



# revision 16
# speedup vs baseline: 21996.3653x; 21996.3653x over previous
"""GNN message passing (2-layer GCN-ish + dense similarity) on 8 trn2 NeuronCores.

Sharding: nodes row-partitioned across 8 cores (1024 rows each).

Design (vs the gather/one-hot baseline this replaces):
- All activations stay feature-transposed [feat-part, node]: no on-device
  transposes anywhere (the host pre-transposes x once).
- The Linear is folded BEFORE the AllGather: y = (x/rowsum) @ W^T is
  computed on own rows only (0.5 GFLOP/core), so the spmm A @ y directly
  produces each layer's linear output and the AllGather moves y.
- The spmm is a dense-adjacency f16 matmul streamed from HBM: A
  [8192 src, 1024 dst] per core, with src order permuted to match the
  chunked-AllGather arrival order (permutation applied host-side for free).
- Row-normalization sums are ones-vector matmuls (partition-direction
  reduce) whose reciprocal folds into one elementwise scale; ELU is
  composed as max(h,0) + min(exp(h),1) - 1 with the bias fused into the
  activation ops.
- The final emb @ emb^T runs in fp8 e4m3 with DoubleRow perf mode (4x f16
  rate; adds ~0.4% global error, validated), ReLU'd to f16 output.
- Every AllGather is split into 4 chunks; each layer computes its two
  dst-halves in separate passes whose tail work (norm + y-GEMM + AG, on
  PE) is spliced into the NEXT pass's matmul stream via chunk-indexed
  callbacks, so collectives launch half a layer early and the in-order
  engine queues never stall on Act/DVE latency.  DMAs are batched (A in
  1MB tiles) and spread across the SP/Act/Pool DGE queues so semaphore
  waits never head-of-line-block a throughput stream.
- reps>1 unrolls the whole computation for dispatch-amortized timing;
  rep r+1's phase-0 (normalize + y0 + AG) is spliced into rep r's final
  matmul stream, pipelining successive iterations.
"""
import sys

sys.path.insert(0, "/opt/trn_rl_repo")

import numpy as np
import ml_dtypes  # noqa: F401

import concourse.bass as bass
import concourse.bacc as bacc
import concourse.mybir as mybir
from concourse import tile
from concourse.tile import add_dep_helper

N = 8192        # nodes
D = 512         # feature dim
C = 8           # cores
NL = N // C     # nodes per core (1024)
NCH = N // 128  # src chunks (64)
NAG = 4         # AllGather chunks per layer
ROWS_AG = NL // NAG   # own rows per AG chunk (256)
CH_AG = NCH // NAG    # src chunks per AG chunk (16)

f32 = mybir.dt.float32
f32r = mybir.dt.float32r
f16 = mybir.dt.float16
f8 = mybir.dt.float8e4

_compiled: dict = {}
ABLATE: set = set()


def _build(reps: int = 1, timing: bool = False):
    """Build the SPMD program.  reps>1 repeats the full computation for
    dispatch-amortized timing.  timing=True replaces collectives with
    equivalent-volume local DMAs (single-core simulatable).

    Structure per layer: two dst-half spmm passes (b=0: dst cols 0-511,
    b=1: 512-1023).  The tail of half b (ELU -> normalize -> y-GEMM -> AG,
    or ELU -> L2 -> embT -> AG for the last layer) has its PE work spliced
    into the NEXT pass's matmul stream via chunk-indexed callbacks, so the
    in-order PE queue never stalls on Act/DVE latency and each AllGather
    launches half a layer early (hidden behind the other half's spmm).
    """
    from concourse import library_config

    nc = bacc.Bacc("TRN2", target_bir_lowering=False, debug=False, num_devices=C)

    xT = nc.declare_dram_parameter("xT", [D, NL], f32, isOutput=False)
    a0 = nc.declare_dram_parameter("a0", [NCH, 128, 512], f16, isOutput=False)
    a1 = nc.declare_dram_parameter("a1", [NCH, 128, 512], f16, isOutput=False)
    wt = nc.declare_dram_parameter("wt", [4, 128, 512], f16, isOutput=False)
    bcol = nc.declare_dram_parameter("bcol", [128, 4], f32, isOutput=False)
    onesf = nc.declare_dram_parameter("onesf", [128, 128], f32, isOutput=False)
    ones16 = nc.declare_dram_parameter("ones16", [128, 128], f16, isOutput=False)
    out = nc.declare_dram_parameter("out", [NL, N], f16, isOutput=True)

    Act = mybir.ActivationFunctionType
    Alu = mybir.AluOpType
    PM = mybir.MatmulPerfMode
    rg = [list(range(C))]

    with tile.TileContext(nc) as tc:
        if not timing:
            nc.gpsimd.load_library(library_config.mlp)
        with (
            tc.tile_pool(name="persist", bufs=1) as pp,
            tc.tile_pool(name="dram", bufs=1, space="DRAM") as dram,
        ):
            wt_sb = pp.tile([128, 4, 512], f16)
            bc_sb = pp.tile([128, 4], f32)
            onf_sb = pp.tile([128, 128], f32)
            on16_sb = pp.tile([128, 128], f16)
            nc.sync.dma_start(out=wt_sb[:], in_=wt.rearrange("c p f -> p c f"))
            nc.sync.dma_start(out=bc_sb[:], in_=bcol[:])
            nc.sync.dma_start(out=onf_sb[:], in_=onesf[:])
            nc.sync.dma_start(out=on16_sb[:], in_=ones16[:])

            shr = "Local" if timing else "Shared"
            ag_y = [
                [dram.tile([ROWS_AG, D], f16, name=f"agy{l}_{g}") for g in range(NAG)]
                for l in range(2)
            ]
            ag_e = [dram.tile([D, ROWS_AG], f8, name=f"age{g}") for g in range(NAG)]

            def all_gather(src_t, dst_t, rows):
                if timing:
                    last = None
                    for r in range(2):
                        last = nc.sync.dma_start(
                            out=dst_t[r * rows : (r + 1) * rows], in_=src_t[:]
                        )
                    return last
                return nc.gpsimd.collective_compute(
                    "AllGather",
                    Alu.bypass,
                    ins=[src_t.opt()],
                    outs=[dst_t.opt()],
                    replica_groups=rg,
                )

            with (
                tc.tile_pool(name="ychk", bufs=4) as ychk,
                tc.tile_pool(name="astr", bufs=2) as astr,
                tc.tile_pool(name="x0p", bufs=1) as x0p,
                tc.tile_pool(name="xtp", bufs=2) as xtp,
                tc.tile_pool(name="f32p", bufs=1) as f32p,
                tc.tile_pool(name="smp", bufs=2) as smp,
                tc.tile_pool(name="ysb", bufs=2) as ysbp,
                tc.tile_pool(name="embp", bufs=1) as embp,
                tc.tile_pool(name="obp", bufs=3) as obp,
                tc.tile_pool(name="ps", bufs=4, space="PSUM") as ps,
            ):
                def make_state(rep):
                    st = {}
                    st["ccs"] = [[None] * NAG for _ in range(2)]
                    st["cce"] = [None] * NAG
                    st["ych"] = [[None] * NAG for _ in range(2)]
                    st["embA"] = [None] * NAG
                    # Shared collective-output buffers: one writer inst each,
                    # so allocate fresh per rep.
                    st["yfull"] = [
                        [
                            dram.tile(
                                [C * ROWS_AG, D], f16, addr_space=shr,
                                name=f"yf{l}_{g}_{rep}",
                            )
                            for g in range(NAG)
                        ]
                        for l in range(2)
                    ]
                    st["efull"] = [
                        dram.tile(
                            [C * D, ROWS_AG], f8, addr_space=shr,
                            name=f"ef{g}_{rep}",
                        )
                        for g in range(NAG)
                    ]
                    return st

                def norm_half(xt_ap4, isf32, name):
                    """Partition-reduce rowsum -> reciprocal -> scaled copy."""
                    rs = ps.tile([128, 2, 512], f32, tag="ps", name=f"rs{name}")
                    lhs = onf_sb if isf32 else on16_sb
                    for fc in range(4):
                        nc.tensor.matmul(
                            rs[:, 0, :],
                            lhsT=lhs[:],
                            rhs=xt_ap4(fc),
                            start=(fc == 0),
                            stop=(fc == 3),
                        )
                    sm = smp.tile([128, 512], f32, tag="sm", name=f"sm{name}")
                    nc.vector.tensor_scalar_add(sm[:], rs[:, 0, :], 1e-4)
                    rr = smp.tile([128, 512], f32, tag="rr", name=f"rr{name}")
                    nc.vector.reciprocal(rr[:], sm[:])
                    xn = xtp.tile([128, 4, 512], f16, tag="x", name=f"xn{name}")
                    for fc in range(4):
                        nc.vector.tensor_tensor(
                            out=xn[:, fc, :], in0=xt_ap4(fc), in1=rr[:],
                            op=Alu.mult,
                        )
                    return xn

                def y_half(st, xn, b, layer_dst, name):
                    """y-GEMM for own rows b*512..b*512+512 -> AG g=2b,2b+1."""
                    yplo = ps.tile([128, 2, 512], f32, tag="ps", name=f"yl{name}")
                    yphi = ps.tile([128, 2, 512], f32, tag="ps", name=f"yh{name}")
                    for blk in range(4):
                        dstp = yplo if blk < 2 else yphi
                        for fi in range(4):
                            nc.tensor.matmul(
                                dstp[:, blk % 2, :],
                                lhsT=xn[:, fi, blk * 128 : (blk + 1) * 128],
                                rhs=wt_sb[:, fi, :],
                                start=(fi == 0),
                                stop=(fi == 3),
                            )
                    ysb = ysbp.tile([128, 4, 512], f16, tag="ysb", name=f"ys{name}")
                    nc.scalar.copy(out=ysb[:, 0:2, :], in_=yplo[:])
                    nc.scalar.copy(out=ysb[:, 2:4, :], in_=yphi[:])
                    for h in range(2):
                        g = 2 * b + h
                        nc.sync.dma_start(
                            out=ag_y[layer_dst][g].rearrange(
                                "(s p) d -> p s d", p=128
                            ),
                            in_=ysb[:, 2 * h : 2 * h + 2, :],
                        )
                        st["ccs"][layer_dst][g] = all_gather(
                            ag_y[layer_dst][g], st["yfull"][layer_dst][g], ROWS_AG
                        )

                def phase0(st, rep):
                    """own x -> per-half norm -> y0 -> AG (sets st.ccs[0])."""
                    x0 = x0p.tile([128, 4, NL], f32, tag="x0", name=f"x0_{rep}")
                    nc.sync.dma_start(
                        out=x0[:], in_=xT.rearrange("(c p) n -> p c n", p=128)
                    )
                    for b in range(2):
                        xn = norm_half(
                            lambda fc, b=b: x0[:, fc, b * 512 : (b + 1) * 512],
                            True, f"p{b}_{rep}",
                        )
                        y_half(st, xn, b, 0, f"p{b}_{rep}")

                def spmm_pass(st, layer, b, cbs):
                    aggL = ps.tile([128, 2, 512], f32, tag="ps", name=f"aL{layer}{b}")
                    aggH = ps.tile([128, 2, 512], f32, tag="ps", name=f"aH{layer}{b}")
                    asrc = (a0 if b == 0 else a1).rearrange("c p f -> p c f")
                    at = None
                    for c in range(NCH):
                        g, u = divmod(c, CH_AG)
                        if b == 0 and u == 0:
                            yt = ychk.tile(
                                [128, CH_AG, 512], f16, tag="y",
                                name=f"ych{layer}{g}",
                            )
                            ld = nc.scalar.dma_start(
                                out=yt[:],
                                in_=st["yfull"][layer][g].rearrange(
                                    "(s p) d -> p s d", p=128
                                ),
                            )
                            if not timing:
                                add_dep_helper(
                                    ld.ins, st["ccs"][layer][g].ins, sync=True,
                                    reason="y chunk reads AG output",
                                )
                            st["ych"][layer][g] = yt
                        if c % 8 == 0:
                            at = astr.tile([128, 8, 512], f16, tag="at")
                            nc.sync.dma_start(
                                out=at[:], in_=asrc[:, c : c + 8, :]
                            )
                        if "spmm" not in ABLATE:
                            yt = st["ych"][layer][g]
                            for fc in range(4):
                                dstp = aggL if fc < 2 else aggH
                                nc.tensor.matmul(
                                    dstp[:, fc % 2, :],
                                    lhsT=yt[:, u, fc * 128 : (fc + 1) * 128],
                                    rhs=at[:, c % 8, :],
                                    start=(c == 0),
                                    stop=(c == NCH - 1),
                                )
                        if c in cbs:
                            cbs[c]()
                    return aggL, aggH

                def elu(aggL, aggH, xt, b):
                    """xt[:, :, b*512:(b+1)*512] = ELU(agg + bias)."""
                    en = f32p.tile([128, 4, 512], f32, tag="en", name=f"en{b}")
                    po = f32p.tile([128, 4, 512], f32, tag="po", name=f"po{b}")
                    for fc in range(4):
                        h = (aggL if fc < 2 else aggH)[:, fc % 2, :]
                        nc.scalar.activation(
                            en[:, fc, :], h, Act.Exp, bias=bc_sb[:, fc : fc + 1]
                        )
                        nc.vector.tensor_scalar_min(en[:, fc, :], en[:, fc, :], 1.0)
                        nc.scalar.activation(
                            po[:, fc, :], h, Act.Relu, bias=bc_sb[:, fc : fc + 1]
                        )
                        nc.vector.tensor_tensor(
                            out=po[:, fc, :], in0=po[:, fc, :], in1=en[:, fc, :],
                            op=Alu.add,
                        )
                        nc.vector.tensor_scalar_add(
                            xt[:, fc, b * 512 : (b + 1) * 512], po[:, fc, :], -1.0
                        )

                def make_y_tail(st, xt, b, layer_dst, rep):
                    """Callbacks producing next layer's y for own-col half b."""
                    hold = {}

                    def cb1():
                        hold["xn"] = norm_half(
                            lambda fc: xt[:, fc, b * 512 : (b + 1) * 512],
                            False, f"t{layer_dst}{b}_{rep}",
                        )

                    def cb2():
                        y_half(st, hold["xn"], b, layer_dst, f"t{layer_dst}{b}_{rep}")

                    return {10: cb1, 18: cb2}

                def emb_tail(st, xt, embT, b):
                    """L2-normalize own-col half b -> fp8 embT -> AG."""
                    sq = f32p.tile([128, 4, 512], f32, tag="en", name=f"sq{b}")
                    for fc in range(4):
                        nc.scalar.activation(
                            sq[:, fc, :], xt[:, fc, b * 512 : (b + 1) * 512],
                            Act.Square,
                        )
                    ssq = ps.tile([128, 2, 512], f32, tag="ps", name=f"ssq{b}")
                    for fc in range(4):
                        nc.tensor.matmul(
                            ssq[:, 0, :],
                            lhsT=onf_sb[:],
                            rhs=sq[:, fc, :],
                            start=(fc == 0),
                            stop=(fc == 3),
                        )
                    nr = smp.tile([128, 512], f32, tag="sm", name=f"nr{b}")
                    nc.vector.tensor_scalar_max(nr[:], ssq[:, 0, :], 1e-24)
                    nc.scalar.activation(nr[:], nr[:], Act.Sqrt)
                    rq = smp.tile([128, 512], f32, tag="rr", name=f"rq{b}")
                    nc.vector.reciprocal(rq[:], nr[:])
                    for fc in range(4):
                        nc.vector.tensor_tensor(
                            out=embT[:, fc, b * 512 : (b + 1) * 512],
                            in0=xt[:, fc, b * 512 : (b + 1) * 512],
                            in1=rq[:],
                            op=Alu.mult,
                        )
                    for h in range(2):
                        g = 2 * b + h
                        nc.sync.dma_start(
                            out=ag_e[g].rearrange("(p c) n -> p c n", p=128),
                            in_=embT[:, :, g * ROWS_AG : (g + 1) * ROWS_AG],
                        )
                        st["cce"][g] = all_gather(ag_e[g], st["efull"][g], D)

                def load_embA(st, g):
                    t = embp.tile([128, 4, 2048], f8, tag=f"eA{g}", name=f"eA{g}")
                    for r in range(C):
                        ld = nc.gpsimd.dma_start(
                            out=t[:, :, r * ROWS_AG : (r + 1) * ROWS_AG],
                            in_=st["efull"][g][r * D : (r + 1) * D].rearrange(
                                "(p c) n -> p c n", p=128
                            ),
                        )
                        if not timing:
                            add_dep_helper(
                                ld.ins, st["cce"][g].ins, sync=True,
                                reason="embA reads AG output",
                            )
                    st["embA"][g] = t

                def final_block(st, embT, gq, m):
                    opsL = ps.tile([128, 2, 512], f32, tag="ps", name=f"oL{gq}{m}")
                    opsH = ps.tile([128, 2, 512], f32, tag="ps", name=f"oH{gq}{m}")
                    if "final" not in ABLATE:
                        for j in range(4):
                            dstp = opsL if j < 2 else opsH
                            for t in range(2):
                                nc.tensor.matmul(
                                    dstp[:, j % 2, :],
                                    lhsT=embT[
                                        :, 2 * t : 2 * t + 2,
                                        m * 128 : (m + 1) * 128,
                                    ],
                                    rhs=st["embA"][gq][
                                        :, 2 * t : 2 * t + 2,
                                        j * 512 : (j + 1) * 512,
                                    ],
                                    perf_mode=PM.DoubleRow,
                                    start=(t == 0),
                                    stop=(t == 1),
                                )
                    ob = obp.tile([128, 8, 256], f16, tag="ob", name=f"ob{gq}{m}")
                    for j in range(4):
                        src = (opsL if j < 2 else opsH)[:, j % 2, :].rearrange(
                            "p (q i) -> p q i", q=2
                        )
                        if j % 2 == 0:
                            nc.scalar.activation(
                                ob[:, 2 * j : 2 * j + 2, :], src, Act.Relu
                            )
                        else:
                            nc.vector.tensor_scalar_max(
                                ob[:, 2 * j : 2 * j + 2, :], src, 0.0
                            )
                    nc.sync.dma_start(
                        out=out[m * 128 : (m + 1) * 128, :].rearrange(
                            "p (r q i) -> p q r i", q=NAG, i=ROWS_AG
                        )[:, gq],
                        in_=ob[:],
                    )

                st = make_state(0)
                phase0(st, 0)
                for rep in range(reps):
                    nxt = make_state(rep + 1) if rep + 1 < reps else None

                    # layer 0
                    xt0 = xtp.tile([128, 4, NL], f16, tag="x", name="xt0")
                    aL, aH = spmm_pass(st, 0, 0, {})
                    elu(aL, aH, xt0, 0)
                    aL, aH = spmm_pass(st, 0, 1, make_y_tail(st, xt0, 0, 1, rep))
                    elu(aL, aH, xt0, 1)
                    tail01 = make_y_tail(st, xt0, 1, 1, rep)

                    # layer 1
                    xt1 = xtp.tile([128, 4, NL], f16, tag="x", name="xt1")
                    embT = embp.tile([128, 4, NL], f8, tag="embT", name="embT")
                    aL, aH = spmm_pass(st, 1, 0, tail01)
                    elu(aL, aH, xt1, 0)

                    def tail10():
                        emb_tail(st, xt1, embT, 0)
                        load_embA(st, 0)
                        load_embA(st, 1)

                    aL, aH = spmm_pass(st, 1, 1, {12: tail10})
                    elu(aL, aH, xt1, 1)

                    # final; splice next rep's phase0 in at block 20 so its
                    # AllGathers complete before its layer-0 pass begins
                    nblk = 0
                    for gq, m in (
                        [(g, m) for g in (0, 1) for m in (0, 1, 2, 3)]
                        + [(g, m) for g in (0, 1) for m in (4, 5, 6, 7)]
                        + [(g, m) for g in (2, 3) for m in range(8)]
                    ):
                        final_block(st, embT, gq, m)
                        nblk += 1
                        if nblk == 8:
                            emb_tail(st, xt1, embT, 1)
                        elif nblk == 12:
                            load_embA(st, 2)
                            load_embA(st, 3)
                        elif nblk == 20 and nxt is not None:
                            phase0(nxt, rep + 1)
                    st = nxt

    nc.finalize()
    return nc


def _preprocess(x, edge_index, edge_weight):
    """Per-core dense adjacency (src order = chunked-AG arrival order)."""
    row = edge_index[0].astype(np.int64)
    col = edge_index[1].astype(np.int64)
    w = edge_weight.astype(np.float32)

    # src permutation: position g*2048 + r*256 + i  <-  global src r*1024 + g*256 + i
    g_idx = np.arange(N)
    r_, loc = g_idx // NL, g_idx % NL
    gg, ii = loc // ROWS_AG, loc % ROWS_AG
    perm_pos = gg * (C * ROWS_AG) + r_ * ROWS_AG + ii

    in_maps = []
    pc = perm_pos[col]
    for k in range(C):
        msk = (row >= k * NL) & (row < (k + 1) * NL)
        A = np.zeros((N, NL), np.float32)
        np.add.at(A, (pc[msk], row[msk] - k * NL), w[msk])
        A = A.astype(np.float16)
        in_maps.append(
            {
                "a0": np.ascontiguousarray(A[:, :512].reshape(NCH, 128, 512)),
                "a1": np.ascontiguousarray(A[:, 512:].reshape(NCH, 128, 512)),
            }
        )
    return in_maps


def _runner(nc):
    """Cached jitted shard_map executor with device-resident zero outputs."""
    import jax
    from jax.sharding import Mesh, PartitionSpec, NamedSharding
    from jax.experimental.shard_map import shard_map
    from concourse.bass2jax import (
        _bass_exec_p,
        partition_id_tensor,
        install_neuronx_cc_hook,
    )

    install_neuronx_cc_hook()
    partition_name = nc.partition_id_tensor.name if nc.partition_id_tensor else None
    in_names, out_names, out_avals, zero_outs = [], [], [], []
    for alloc in nc.m.functions[0].allocations:
        if not isinstance(alloc, mybir.MemoryLocationSet):
            continue
        name = alloc.memorylocations[0].name
        if alloc.kind == "ExternalInput":
            if name != partition_name:
                in_names.append(name)
        elif alloc.kind == "ExternalOutput":
            shape = tuple(alloc.tensor_shape)
            dtype = mybir.dt.np(alloc.dtype)
            out_names.append(name)
            out_avals.append(jax.core.ShapedArray(shape, dtype))
            zero_outs.append(np.zeros(shape, dtype))
    n_params = len(in_names)
    all_in = in_names + out_names
    if partition_name is not None:
        all_in = all_in + [partition_name]

    def _body(*args):
        operands = list(args)
        if partition_name is not None:
            operands.append(partition_id_tensor())
        outs = _bass_exec_p.bind(
            *operands,
            out_avals=tuple(out_avals),
            in_names=tuple(all_in),
            out_names=tuple(out_names),
            lowering_input_output_aliases=(),
            sim_require_finite=True,
            sim_require_nnan=True,
            nc=nc,
        )
        return tuple(outs)

    devices = jax.devices()[:C]
    mesh = Mesh(np.asarray(devices), ("core",))
    n_ops = n_params + len(out_names)
    sharded = jax.jit(
        shard_map(
            _body,
            mesh=mesh,
            in_specs=(PartitionSpec("core"),) * n_ops,
            out_specs=(PartitionSpec("core"),) * len(out_names),
            check_rep=False,
        ),
        keep_unused=True,
    )
    sharding = NamedSharding(mesh, PartitionSpec("core"))
    dev_zeros = [
        jax.device_put(np.zeros((C * z.shape[0], *z.shape[1:]), z.dtype), sharding)
        for z in zero_outs
    ]

    class R:
        pass

    r = R()
    r.in_names, r.out_names, r.n_params = in_names, out_names, n_params
    r.sharded, r.sharding, r.dev_zeros = sharded, sharding, dev_zeros
    r.out_shapes = [tuple(a.shape) for a in out_avals]
    return r


def _put(r, in_maps):
    import jax

    per_core = [[np.asarray(m[name]) for name in r.in_names] for m in in_maps]
    concat_in = [
        np.concatenate([per_core[c][i] for c in range(C)], axis=0)
        for i in range(r.n_params)
    ]
    dev = [jax.device_put(a, r.sharding) for a in concat_in]
    jax.block_until_ready(dev)
    return dev + r.dev_zeros


def _exec(r, dev_args):
    import jax

    outs = r.sharded(*dev_args)
    jax.block_until_ready(outs)
    return outs


def kernel(x, edge_index, edge_weight, W, b):
    x = np.asarray(x, dtype=np.float32)
    edge_index = np.asarray(edge_index)
    edge_weight = np.asarray(edge_weight, dtype=np.float32)
    W = np.asarray(W, dtype=np.float32)
    b = np.asarray(b, dtype=np.float32)

    in_maps = _preprocess(x, edge_index, edge_weight)
    wt = np.ascontiguousarray(W.T.reshape(4, 128, 512)).astype(np.float16)
    bc = np.ascontiguousarray(b.reshape(4, 128).T).astype(np.float32)
    ones = np.ones((128, 128), np.float32)
    for k in range(C):
        in_maps[k]["xT"] = np.ascontiguousarray(x[k * NL : (k + 1) * NL].T)
        in_maps[k]["wt"] = wt
        in_maps[k]["bcol"] = bc
        in_maps[k]["onesf"] = ones
        in_maps[k]["ones16"] = ones.astype(np.float16)

    ent = _compiled.get(1)
    if ent is None:
        nc = _build(1)
        ent = (nc, _runner(nc))
        _compiled[1] = ent
    nc, r = ent
    outs = _exec(r, _put(r, in_maps))
    full = np.asarray(outs[r.out_names.index("out")])
    return full.reshape(C, NL, N).reshape(N, N).astype(np.float32)


# revision 17
# speedup vs baseline: 22410.8534x; 1.0188x over previous
"""GNN message passing (2-layer GCN-ish + dense similarity) on 8 trn2 NeuronCores.

Sharding: nodes row-partitioned across 8 cores (1024 rows each).

Design (vs the gather/one-hot baseline this replaces):
- All activations stay feature-transposed [feat-part, node]: no on-device
  transposes anywhere (the host pre-transposes x once).
- The Linear is folded BEFORE the AllGather: y = (x/rowsum) @ W^T is
  computed on own rows only (0.5 GFLOP/core), so the spmm A @ y directly
  produces each layer's linear output and the AllGather moves y.
- The spmm is a dense-adjacency f16 matmul streamed from HBM: A
  [8192 src, 1024 dst] per core, with src order permuted to match the
  chunked-AllGather arrival order (permutation applied host-side for free).
- Row-normalization sums are ones-vector matmuls (partition-direction
  reduce) whose reciprocal folds into one elementwise scale; ELU is
  composed as max(h,0) + min(exp(h),1) - 1 with the bias fused into the
  activation ops.
- The final emb @ emb^T runs in fp8 e4m3 with DoubleRow perf mode (4x f16
  rate; adds ~0.4% global error, validated), ReLU'd to f16 output.
- Every AllGather is split into 4 chunks; each layer computes its two
  dst-halves in separate passes whose tail work (norm + y-GEMM + AG, on
  PE) is spliced into the NEXT pass's matmul stream via chunk-indexed
  callbacks, so collectives launch half a layer early and the in-order
  engine queues never stall on Act/DVE latency.  DMAs are batched (A in
  1MB tiles) and spread across the SP/Act/Pool DGE queues so semaphore
  waits never head-of-line-block a throughput stream.
- reps>1 unrolls the whole computation for dispatch-amortized timing;
  rep r+1's phase-0 (normalize + y0 + AG) is spliced into rep r's final
  matmul stream, pipelining successive iterations.
"""
import sys

sys.path.insert(0, "/opt/trn_rl_repo")

import numpy as np
import ml_dtypes  # noqa: F401

import concourse.bass as bass
import concourse.bacc as bacc
import concourse.mybir as mybir
from concourse import tile
from concourse.tile import add_dep_helper

N = 8192        # nodes
D = 512         # feature dim
C = 8           # cores
NL = N // C     # nodes per core (1024)
NCH = N // 128  # src chunks (64)
NAG = 4         # AllGather chunks per layer
ROWS_AG = NL // NAG   # own rows per AG chunk (256)
CH_AG = NCH // NAG    # src chunks per AG chunk (16)

f32 = mybir.dt.float32
f32r = mybir.dt.float32r
f16 = mybir.dt.float16
f8 = mybir.dt.float8e4

_compiled: dict = {}
ABLATE: set = set()


def _build(reps: int = 1, timing: bool = False):
    """Build the SPMD program.  reps>1 repeats the full computation for
    dispatch-amortized timing.  timing=True replaces collectives with
    equivalent-volume local DMAs (single-core simulatable).

    Structure per layer: two dst-half spmm passes (b=0: dst cols 0-511,
    b=1: 512-1023).  The tail of half b (ELU -> normalize -> y-GEMM -> AG,
    or ELU -> L2 -> embT -> AG for the last layer) has its PE work spliced
    into the NEXT pass's matmul stream via chunk-indexed callbacks, so the
    in-order PE queue never stalls on Act/DVE latency and each AllGather
    launches half a layer early (hidden behind the other half's spmm).
    """
    from concourse import library_config

    nc = bacc.Bacc("TRN2", target_bir_lowering=False, debug=False, num_devices=C)

    xT = nc.declare_dram_parameter("xT", [D, NL], f32, isOutput=False)
    a0 = nc.declare_dram_parameter("a0", [NCH, 128, 512], f16, isOutput=False)
    a1 = nc.declare_dram_parameter("a1", [NCH, 128, 512], f16, isOutput=False)
    wt = nc.declare_dram_parameter("wt", [4, 128, 512], f16, isOutput=False)
    bcol = nc.declare_dram_parameter("bcol", [128, 4], f32, isOutput=False)
    onesf = nc.declare_dram_parameter("onesf", [128, 128], f32, isOutput=False)
    ones16 = nc.declare_dram_parameter("ones16", [128, 128], f16, isOutput=False)
    out = nc.declare_dram_parameter("out", [NL, N], f16, isOutput=True)

    Act = mybir.ActivationFunctionType
    Alu = mybir.AluOpType
    PM = mybir.MatmulPerfMode
    rg = [list(range(C))]

    with tile.TileContext(nc) as tc:
        if not timing:
            nc.gpsimd.load_library(library_config.mlp)
        with (
            tc.tile_pool(name="persist", bufs=1) as pp,
            tc.tile_pool(name="dram", bufs=1, space="DRAM") as dram,
        ):
            wt_sb = pp.tile([128, 4, 512], f16)
            bc_sb = pp.tile([128, 4], f32)
            onf_sb = pp.tile([128, 128], f32)
            on16_sb = pp.tile([128, 128], f16)
            nc.sync.dma_start(out=wt_sb[:], in_=wt.rearrange("c p f -> p c f"))
            nc.sync.dma_start(out=bc_sb[:], in_=bcol[:])
            nc.sync.dma_start(out=onf_sb[:], in_=onesf[:])
            nc.sync.dma_start(out=on16_sb[:], in_=ones16[:])

            shr = "Local" if timing else "Shared"
            ag_y = [
                [dram.tile([ROWS_AG, D], f16, name=f"agy{l}_{g}") for g in range(NAG)]
                for l in range(2)
            ]
            ag_e = [dram.tile([D, ROWS_AG], f8, name=f"age{g}") for g in range(NAG)]

            def all_gather(src_t, dst_t, rows):
                if timing:
                    last = None
                    for r in range(2):
                        last = nc.sync.dma_start(
                            out=dst_t[r * rows : (r + 1) * rows], in_=src_t[:]
                        )
                    return last
                return nc.gpsimd.collective_compute(
                    "AllGather",
                    Alu.bypass,
                    ins=[src_t.opt()],
                    outs=[dst_t.opt()],
                    replica_groups=rg,
                )

            with (
                tc.tile_pool(name="ychk", bufs=4) as ychk,
                tc.tile_pool(name="astr", bufs=2) as astr,
                tc.tile_pool(name="x0p", bufs=1) as x0p,
                tc.tile_pool(name="xtp", bufs=2) as xtp,
                tc.tile_pool(name="f32p", bufs=1) as f32p,
                tc.tile_pool(name="smp", bufs=2) as smp,
                tc.tile_pool(name="ysb", bufs=2) as ysbp,
                tc.tile_pool(name="embp", bufs=1) as embp,
                tc.tile_pool(name="obp", bufs=3) as obp,
                tc.tile_pool(name="ps", bufs=4, space="PSUM") as ps,
            ):
                def make_state(rep):
                    st = {}
                    st["ccs"] = [[None] * NAG for _ in range(2)]
                    st["cce"] = [None] * NAG
                    st["ych"] = [[None] * NAG for _ in range(2)]
                    st["embA"] = [None] * NAG
                    # Shared collective-output buffers: one writer inst each,
                    # so allocate fresh per rep.
                    st["yfull"] = [
                        [
                            dram.tile(
                                [C * ROWS_AG, D], f16, addr_space=shr,
                                name=f"yf{l}_{g}_{rep}",
                            )
                            for g in range(NAG)
                        ]
                        for l in range(2)
                    ]
                    st["efull"] = [
                        dram.tile(
                            [C * D, ROWS_AG], f8, addr_space=shr,
                            name=f"ef{g}_{rep}",
                        )
                        for g in range(NAG)
                    ]
                    return st

                def norm_half(xt_ap4, isf32, name):
                    """Partition-reduce rowsum -> reciprocal -> scaled copy."""
                    rs = ps.tile([128, 2, 512], f32, tag="ps", name=f"rs{name}")
                    lhs = onf_sb if isf32 else on16_sb
                    for fc in range(4):
                        nc.tensor.matmul(
                            rs[:, 0, :],
                            lhsT=lhs[:],
                            rhs=xt_ap4(fc),
                            start=(fc == 0),
                            stop=(fc == 3),
                        )
                    sm = smp.tile([128, 512], f32, tag="sm", name=f"sm{name}")
                    nc.vector.tensor_scalar_add(sm[:], rs[:, 0, :], 1e-4)
                    rr = smp.tile([128, 512], f32, tag="rr", name=f"rr{name}")
                    nc.vector.reciprocal(rr[:], sm[:])
                    xn = xtp.tile([128, 4, 512], f16, tag="x", name=f"xn{name}")
                    for fc in range(4):
                        nc.vector.tensor_tensor(
                            out=xn[:, fc, :], in0=xt_ap4(fc), in1=rr[:],
                            op=Alu.mult,
                        )
                    return xn

                def y_half(st, xn, b, layer_dst, name, agdma=None):
                    """y-GEMM for own rows b*512..b*512+512 -> AG g=2b,2b+1."""
                    yplo = ps.tile([128, 2, 512], f32, tag="ps", name=f"yl{name}")
                    yphi = ps.tile([128, 2, 512], f32, tag="ps", name=f"yh{name}")
                    for blk in range(4):
                        dstp = yplo if blk < 2 else yphi
                        for fi in range(4):
                            nc.tensor.matmul(
                                dstp[:, blk % 2, :],
                                lhsT=xn[:, fi, blk * 128 : (blk + 1) * 128],
                                rhs=wt_sb[:, fi, :],
                                start=(fi == 0),
                                stop=(fi == 3),
                            )
                    ysb = ysbp.tile([128, 4, 512], f16, tag="ysb", name=f"ys{name}")
                    nc.scalar.copy(out=ysb[:, 0:2, :], in_=yplo[:])
                    nc.scalar.copy(out=ysb[:, 2:4, :], in_=yphi[:])
                    for h in range(2):
                        g = 2 * b + h
                        (agdma or nc.sync).dma_start(
                            out=ag_y[layer_dst][g].rearrange(
                                "(s p) d -> p s d", p=128
                            ),
                            in_=ysb[:, 2 * h : 2 * h + 2, :],
                        )
                        st["ccs"][layer_dst][g] = all_gather(
                            ag_y[layer_dst][g], st["yfull"][layer_dst][g], ROWS_AG
                        )

                def phase0(st, rep):
                    """own x -> per-half norm -> y0 -> AG (sets st.ccs[0]).
                    DMAs ride the Pool queue: phase0 is spliced into the
                    previous rep's final phase, whose out-writes jam SP."""
                    x0 = x0p.tile([128, 4, NL], f32, tag="x0", name=f"x0_{rep}")
                    nc.gpsimd.dma_start(
                        out=x0[:], in_=xT.rearrange("(c p) n -> p c n", p=128)
                    )
                    for b in range(2):
                        xn = norm_half(
                            lambda fc, b=b: x0[:, fc, b * 512 : (b + 1) * 512],
                            True, f"p{b}_{rep}",
                        )
                        y_half(st, xn, b, 0, f"p{b}_{rep}", agdma=nc.gpsimd)

                def spmm_pass(st, layer, b, cbs):
                    aggL = ps.tile([128, 2, 512], f32, tag="ps", name=f"aL{layer}{b}")
                    aggH = ps.tile([128, 2, 512], f32, tag="ps", name=f"aH{layer}{b}")
                    asrc = (a0 if b == 0 else a1).rearrange("c p f -> p c f")
                    at = None
                    for c in range(NCH):
                        g, u = divmod(c, CH_AG)
                        if b == 0 and u == 0:
                            yt = ychk.tile(
                                [128, CH_AG, 512], f16, tag="y",
                                name=f"ych{layer}{g}",
                            )
                            ld = nc.scalar.dma_start(
                                out=yt[:],
                                in_=st["yfull"][layer][g].rearrange(
                                    "(s p) d -> p s d", p=128
                                ),
                            )
                            if not timing:
                                add_dep_helper(
                                    ld.ins, st["ccs"][layer][g].ins, sync=True,
                                    reason="y chunk reads AG output",
                                )
                            st["ych"][layer][g] = yt
                        if c % 8 == 0:
                            at = astr.tile([128, 8, 512], f16, tag="at")
                            nc.sync.dma_start(
                                out=at[:], in_=asrc[:, c : c + 8, :]
                            )
                        if "spmm" not in ABLATE:
                            yt = st["ych"][layer][g]
                            for fc in range(4):
                                dstp = aggL if fc < 2 else aggH
                                nc.tensor.matmul(
                                    dstp[:, fc % 2, :],
                                    lhsT=yt[:, u, fc * 128 : (fc + 1) * 128],
                                    rhs=at[:, c % 8, :],
                                    start=(c == 0),
                                    stop=(c == NCH - 1),
                                )
                        if c in cbs:
                            cbs[c]()
                    return aggL, aggH

                def elu(aggL, aggH, xt, b):
                    """xt[:, :, b*512:(b+1)*512] = ELU(agg + bias)."""
                    en = f32p.tile([128, 4, 512], f32, tag="en", name=f"en{b}")
                    po = f32p.tile([128, 4, 512], f32, tag="po", name=f"po{b}")
                    for fc in range(4):
                        h = (aggL if fc < 2 else aggH)[:, fc % 2, :]
                        nc.scalar.activation(
                            en[:, fc, :], h, Act.Exp, bias=bc_sb[:, fc : fc + 1]
                        )
                        nc.vector.tensor_scalar_min(en[:, fc, :], en[:, fc, :], 1.0)
                        nc.scalar.activation(
                            po[:, fc, :], h, Act.Relu, bias=bc_sb[:, fc : fc + 1]
                        )
                        nc.vector.tensor_tensor(
                            out=po[:, fc, :], in0=po[:, fc, :], in1=en[:, fc, :],
                            op=Alu.add,
                        )
                        nc.vector.tensor_scalar_add(
                            xt[:, fc, b * 512 : (b + 1) * 512], po[:, fc, :], -1.0
                        )

                def make_y_tail(st, xt, b, layer_dst, rep):
                    """Callbacks producing next layer's y for own-col half b."""
                    hold = {}

                    def cb1():
                        hold["xn"] = norm_half(
                            lambda fc: xt[:, fc, b * 512 : (b + 1) * 512],
                            False, f"t{layer_dst}{b}_{rep}",
                        )

                    def cb2():
                        y_half(st, hold["xn"], b, layer_dst, f"t{layer_dst}{b}_{rep}")

                    return {8: cb1, 15: cb2}

                def emb_tail(st, xt, embT, b):
                    """L2-normalize own-col half b -> fp8 embT -> AG."""
                    sq = f32p.tile([128, 4, 512], f32, tag="en", name=f"sq{b}")
                    for fc in range(4):
                        nc.scalar.activation(
                            sq[:, fc, :], xt[:, fc, b * 512 : (b + 1) * 512],
                            Act.Square,
                        )
                    ssq = ps.tile([128, 2, 512], f32, tag="ps", name=f"ssq{b}")
                    for fc in range(4):
                        nc.tensor.matmul(
                            ssq[:, 0, :],
                            lhsT=onf_sb[:],
                            rhs=sq[:, fc, :],
                            start=(fc == 0),
                            stop=(fc == 3),
                        )
                    nr = smp.tile([128, 512], f32, tag="sm", name=f"nr{b}")
                    nc.vector.tensor_scalar_max(nr[:], ssq[:, 0, :], 1e-24)
                    nc.scalar.activation(nr[:], nr[:], Act.Sqrt)
                    rq = smp.tile([128, 512], f32, tag="rr", name=f"rq{b}")
                    nc.vector.reciprocal(rq[:], nr[:])
                    for fc in range(4):
                        nc.vector.tensor_tensor(
                            out=embT[:, fc, b * 512 : (b + 1) * 512],
                            in0=xt[:, fc, b * 512 : (b + 1) * 512],
                            in1=rq[:],
                            op=Alu.mult,
                        )
                    for h in range(2):
                        g = 2 * b + h
                        nc.sync.dma_start(
                            out=ag_e[g].rearrange("(p c) n -> p c n", p=128),
                            in_=embT[:, :, g * ROWS_AG : (g + 1) * ROWS_AG],
                        )
                        st["cce"][g] = all_gather(ag_e[g], st["efull"][g], D)

                def load_embA(st, g):
                    t = embp.tile([128, 4, 2048], f8, tag=f"eA{g}", name=f"eA{g}")
                    for r in range(C):
                        ld = nc.gpsimd.dma_start(
                            out=t[:, :, r * ROWS_AG : (r + 1) * ROWS_AG],
                            in_=st["efull"][g][r * D : (r + 1) * D].rearrange(
                                "(p c) n -> p c n", p=128
                            ),
                        )
                        if not timing:
                            add_dep_helper(
                                ld.ins, st["cce"][g].ins, sync=True,
                                reason="embA reads AG output",
                            )
                    st["embA"][g] = t

                def final_block(st, embT, gq, m):
                    opsL = ps.tile([128, 2, 512], f32, tag="ps", name=f"oL{gq}{m}")
                    opsH = ps.tile([128, 2, 512], f32, tag="ps", name=f"oH{gq}{m}")
                    if "final" not in ABLATE:
                        for j in range(4):
                            dstp = opsL if j < 2 else opsH
                            for t in range(2):
                                nc.tensor.matmul(
                                    dstp[:, j % 2, :],
                                    lhsT=embT[
                                        :, 2 * t : 2 * t + 2,
                                        m * 128 : (m + 1) * 128,
                                    ],
                                    rhs=st["embA"][gq][
                                        :, 2 * t : 2 * t + 2,
                                        j * 512 : (j + 1) * 512,
                                    ],
                                    perf_mode=PM.DoubleRow,
                                    start=(t == 0),
                                    stop=(t == 1),
                                )
                    ob = obp.tile([128, 8, 256], f16, tag="ob", name=f"ob{gq}{m}")
                    for j in range(4):
                        src = (opsL if j < 2 else opsH)[:, j % 2, :].rearrange(
                            "p (q i) -> p q i", q=2
                        )
                        if j % 2 == 0:
                            nc.scalar.activation(
                                ob[:, 2 * j : 2 * j + 2, :], src, Act.Relu
                            )
                        else:
                            nc.vector.tensor_scalar_max(
                                ob[:, 2 * j : 2 * j + 2, :], src, 0.0
                            )
                    eng = nc.sync if (gq + m) % 2 == 0 else nc.scalar
                    eng.dma_start(
                        out=out[m * 128 : (m + 1) * 128, :].rearrange(
                            "p (r q i) -> p q r i", q=NAG, i=ROWS_AG
                        )[:, gq],
                        in_=ob[:],
                    )

                st = make_state(0)
                phase0(st, 0)
                for rep in range(reps):
                    nxt = make_state(rep + 1) if rep + 1 < reps else None

                    # layer 0
                    xt0 = xtp.tile([128, 4, NL], f16, tag="x", name="xt0")
                    aL, aH = spmm_pass(st, 0, 0, {})
                    elu(aL, aH, xt0, 0)
                    aL, aH = spmm_pass(st, 0, 1, make_y_tail(st, xt0, 0, 1, rep))
                    elu(aL, aH, xt0, 1)
                    tail01 = make_y_tail(st, xt0, 1, 1, rep)

                    # layer 1
                    xt1 = xtp.tile([128, 4, NL], f16, tag="x", name="xt1")
                    embT = embp.tile([128, 4, NL], f8, tag="embT", name="embT")
                    aL, aH = spmm_pass(st, 1, 0, tail01)
                    elu(aL, aH, xt1, 0)

                    def tail10():
                        emb_tail(st, xt1, embT, 0)
                        load_embA(st, 0)
                        load_embA(st, 1)

                    aL, aH = spmm_pass(st, 1, 1, {12: tail10})
                    elu(aL, aH, xt1, 1)

                    # final; splice next rep's phase0 in at block 20 so its
                    # AllGathers complete before its layer-0 pass begins
                    nblk = 0
                    for gq, m in (
                        [(g, m) for g in (0, 1) for m in (0, 1, 2, 3)]
                        + [(g, m) for g in (0, 1) for m in (4, 5, 6, 7)]
                        + [(g, m) for g in (2, 3) for m in range(8)]
                    ):
                        final_block(st, embT, gq, m)
                        nblk += 1
                        if nblk == 8:
                            emb_tail(st, xt1, embT, 1)
                        elif nblk == 12:
                            load_embA(st, 2)
                            load_embA(st, 3)
                        elif nblk == 20 and nxt is not None:
                            phase0(nxt, rep + 1)
                    st = nxt

    nc.finalize()
    return nc


def _preprocess(x, edge_index, edge_weight):
    """Per-core dense adjacency (src order = chunked-AG arrival order)."""
    row = edge_index[0].astype(np.int64)
    col = edge_index[1].astype(np.int64)
    w = edge_weight.astype(np.float32)

    # src permutation: position g*2048 + r*256 + i  <-  global src r*1024 + g*256 + i
    g_idx = np.arange(N)
    r_, loc = g_idx // NL, g_idx % NL
    gg, ii = loc // ROWS_AG, loc % ROWS_AG
    perm_pos = gg * (C * ROWS_AG) + r_ * ROWS_AG + ii

    in_maps = []
    pc = perm_pos[col]
    for k in range(C):
        msk = (row >= k * NL) & (row < (k + 1) * NL)
        A = np.zeros((N, NL), np.float32)
        np.add.at(A, (pc[msk], row[msk] - k * NL), w[msk])
        A = A.astype(np.float16)
        in_maps.append(
            {
                "a0": np.ascontiguousarray(A[:, :512].reshape(NCH, 128, 512)),
                "a1": np.ascontiguousarray(A[:, 512:].reshape(NCH, 128, 512)),
            }
        )
    return in_maps


def _runner(nc):
    """Cached jitted shard_map executor with device-resident zero outputs."""
    import jax
    from jax.sharding import Mesh, PartitionSpec, NamedSharding
    from jax.experimental.shard_map import shard_map
    from concourse.bass2jax import (
        _bass_exec_p,
        partition_id_tensor,
        install_neuronx_cc_hook,
    )

    install_neuronx_cc_hook()
    partition_name = nc.partition_id_tensor.name if nc.partition_id_tensor else None
    in_names, out_names, out_avals, zero_outs = [], [], [], []
    for alloc in nc.m.functions[0].allocations:
        if not isinstance(alloc, mybir.MemoryLocationSet):
            continue
        name = alloc.memorylocations[0].name
        if alloc.kind == "ExternalInput":
            if name != partition_name:
                in_names.append(name)
        elif alloc.kind == "ExternalOutput":
            shape = tuple(alloc.tensor_shape)
            dtype = mybir.dt.np(alloc.dtype)
            out_names.append(name)
            out_avals.append(jax.core.ShapedArray(shape, dtype))
            zero_outs.append(np.zeros(shape, dtype))
    n_params = len(in_names)
    all_in = in_names + out_names
    if partition_name is not None:
        all_in = all_in + [partition_name]

    def _body(*args):
        operands = list(args)
        if partition_name is not None:
            operands.append(partition_id_tensor())
        outs = _bass_exec_p.bind(
            *operands,
            out_avals=tuple(out_avals),
            in_names=tuple(all_in),
            out_names=tuple(out_names),
            lowering_input_output_aliases=(),
            sim_require_finite=True,
            sim_require_nnan=True,
            nc=nc,
        )
        return tuple(outs)

    devices = jax.devices()[:C]
    mesh = Mesh(np.asarray(devices), ("core",))
    n_ops = n_params + len(out_names)
    sharded = jax.jit(
        shard_map(
            _body,
            mesh=mesh,
            in_specs=(PartitionSpec("core"),) * n_ops,
            out_specs=(PartitionSpec("core"),) * len(out_names),
            check_rep=False,
        ),
        keep_unused=True,
    )
    sharding = NamedSharding(mesh, PartitionSpec("core"))
    dev_zeros = [
        jax.device_put(np.zeros((C * z.shape[0], *z.shape[1:]), z.dtype), sharding)
        for z in zero_outs
    ]

    class R:
        pass

    r = R()
    r.in_names, r.out_names, r.n_params = in_names, out_names, n_params
    r.sharded, r.sharding, r.dev_zeros = sharded, sharding, dev_zeros
    r.out_shapes = [tuple(a.shape) for a in out_avals]
    return r


def _put(r, in_maps):
    import jax

    per_core = [[np.asarray(m[name]) for name in r.in_names] for m in in_maps]
    concat_in = [
        np.concatenate([per_core[c][i] for c in range(C)], axis=0)
        for i in range(r.n_params)
    ]
    dev = [jax.device_put(a, r.sharding) for a in concat_in]
    jax.block_until_ready(dev)
    return dev + r.dev_zeros


def _exec(r, dev_args):
    import jax

    outs = r.sharded(*dev_args)
    jax.block_until_ready(outs)
    return outs


def kernel(x, edge_index, edge_weight, W, b):
    x = np.asarray(x, dtype=np.float32)
    edge_index = np.asarray(edge_index)
    edge_weight = np.asarray(edge_weight, dtype=np.float32)
    W = np.asarray(W, dtype=np.float32)
    b = np.asarray(b, dtype=np.float32)

    in_maps = _preprocess(x, edge_index, edge_weight)
    wt = np.ascontiguousarray(W.T.reshape(4, 128, 512)).astype(np.float16)
    bc = np.ascontiguousarray(b.reshape(4, 128).T).astype(np.float32)
    ones = np.ones((128, 128), np.float32)
    for k in range(C):
        in_maps[k]["xT"] = np.ascontiguousarray(x[k * NL : (k + 1) * NL].T)
        in_maps[k]["wt"] = wt
        in_maps[k]["bcol"] = bc
        in_maps[k]["onesf"] = ones
        in_maps[k]["ones16"] = ones.astype(np.float16)

    ent = _compiled.get(1)
    if ent is None:
        nc = _build(1)
        ent = (nc, _runner(nc))
        _compiled[1] = ent
    nc, r = ent
    outs = _exec(r, _put(r, in_maps))
    full = np.asarray(outs[r.out_names.index("out")])
    return full.reshape(C, NL, N).reshape(N, N).astype(np.float32)
